# revision 35
# baseline (speedup 1.0000x reference)
"""GNN ensemble MoE-routing kernel for Trainium2 (8 NeuronCores).

Reference computes all 8 expert MLPs for every sample then selects one
(8x wasted FLOPs). This kernel routes on the host instead: samples are
gathered per expert, and core c runs ONLY expert c's MLP over the
samples routed to it (expert-parallel sharding).

Math folding (exact):
  lat = eps*sigma_c + mu_c  =>  lat @ W1_c = eps @ (sigma_c*W1_c) + mu_c@W1_c
so the device computes  sigmoid(eps @ W1p + b1p) @ W2 + b2  with
  W1p = sigma_c * W1_c,  b1p = b1_c + mu_c @ W1_c  (folded on host).

Precision/perf layout (rel-err budget 2e-2; measured ~1.2e-2):
  mm1 runs in fp8 e4m3 with perf_mode=DoubleRow (2 contraction rows
  packed per PE cell -> half the matmul count of bf16). eps quantizes
  to e4m3 raw (std 1.0); W1p is scaled x64 before quantization so its
  ~0.03-std values stay in e4m3's normal range, and the 1/64 rescale
  folds into the sigmoid activation's scale operand. mm2 runs in bf16
  (same PE speed as f32r, half the DMA bytes). y writes back as bf16.

Device layout: features on SBUF partitions, samples on the free axis.
DoubleRow operands are [128, K_blocks, free] with contraction index
k = block*128 + partition; host tensors pack as
reshape(blocks,128,cols).transpose(1,0,2) -> contiguous multi-KB DMA
descriptors. Chunk 0's eps columns are packed as their own dram
tensor so the first transfer is small and fully contiguous.

Startup hiding (the NEFF spends ~6.5us in a fixed semaphore/iram
preamble, then ~4-5us moving the first inputs):
  - The PE's first 8 real matmuls would run ~2x slow (p-state ramp,
    ~3us to full clock), so 8 throwaway DoubleRow matmuls over a
    memset scratch tile ramp the array while the real inputs land.
  - The first Sigmoid pays a ~1.3us ACT_TABLE_LOAD; a throwaway
    activation preloads the table during the head.
mm1 iterates g-outer (first pass needs only half of w1/eps), mm2
oc-outer so bias-add + writeback stream during mm2.
"""

from contextlib import ExitStack

import ml_dtypes
import numpy as np

import concourse.bass as bass
import concourse.tile as tile
from concourse import bacc, mybir
from concourse.bass_utils import run_bass_kernel_spmd

NB_COMP = 8
LAT_DIM = 512
NB_NEUR = 1024
OUT_DIM = 512
N_CORES = 8

F32 = mybir.dt.float32
BF16 = mybir.dt.bfloat16
FP8 = mybir.dt.float8e4
SIG = mybir.ActivationFunctionType.Sigmoid
DR = mybir.MatmulPerfMode.DoubleRow

E4M3 = ml_dtypes.float8_e4m3
NPBF16 = ml_dtypes.bfloat16
W1_SCALE = 64.0
N_WARM_MM = 8

KB1 = LAT_DIM // 128   # 4 contraction blocks for mm1
MC1 = NB_NEUR // 128   # 8 output tiles for mm1
KC2 = NB_NEUR // 128   # 8 contraction blocks for mm2
MC2 = OUT_DIM // 128   # 4 output tiles for mm2

_program_cache = {}


def _make_chunks(k_cap):
    """Near-equal chunks, multiples of 16, each <=512 (PSUM bank / moving
    dim limit) and >=256 when possible (full-rate floor). (A small last
    chunk to shorten the writeback tail was tried and measured worse.)"""
    n_chunks = -(-k_cap // 512)
    base = (k_cap // n_chunks) // 16 * 16
    sizes = [base] * n_chunks
    sizes[0] += k_cap - base * n_chunks
    if n_chunks >= 3:
        # Shrink chunk 0 to 256: its x0 DMA + mm1 gate the pipeline fill,
        # and the surplus redistributes into the middle chunks (<=512).
        surplus = sizes[0] - 256
        sizes[0] = 256
        i = 1
        while surplus > 0 and i < n_chunks - 1:
            add = min(512 - sizes[i], surplus)
            sizes[i] += add
            surplus -= add
            i += 1
        sizes[-1] += surplus
    chunks = []
    n0 = 0
    for ns in sizes:
        chunks.append((n0, ns))
        n0 += ns
    return chunks


def _build_program(k_cap):
    """One-expert MLP over k_cap samples; same program runs SPMD on all 8 cores."""
    chunks = _make_chunks(k_cap)
    ns0 = chunks[0][1]
    k_rest = k_cap - ns0

    nc = bacc.Bacc(
        "TRN2",
        target_bir_lowering=False,
        debug=False,
        enable_asserts=False,
        num_devices=N_CORES,
    )
    eps0 = nc.dram_tensor("eps0", [128, KB1, ns0], FP8, kind="ExternalInput").ap()
    epsr = nc.dram_tensor("epsr", [128, KB1, max(k_rest, 16)], FP8, kind="ExternalInput").ap()
    w1 = nc.dram_tensor("w1", [128, KB1, NB_NEUR], FP8, kind="ExternalInput").ap()
    w2 = nc.dram_tensor("w2", [128, KC2, OUT_DIM], BF16, kind="ExternalInput").ap()
    bias = nc.dram_tensor("bias", [128, MC1 + MC2], F32, kind="ExternalInput").ap()
    yT = nc.dram_tensor("yT", [OUT_DIM, k_cap], BF16, kind="ExternalOutput").ap()

    with tile.TileContext(nc) as tc, ExitStack() as ctx:
        wpool = ctx.enter_context(tc.tile_pool(name="weights", bufs=1))
        hpool = ctx.enter_context(tc.tile_pool(name="h", bufs=2))
        ypool = ctx.enter_context(tc.tile_pool(name="y", bufs=10))
        # One shared pool holding all 8 PSUM banks; mm1 keeps 8 accumulators
        # live, mm2 4, cycling through the same slots.
        pspool = ctx.enter_context(tc.tile_pool(name="ps", bufs=8, space="PSUM"))

        w1t = [
            wpool.tile([128, 2, NB_NEUR], FP8, tag=f"w1{g}", name=f"w1t{g}")
            for g in range(2)
        ]
        x0t = wpool.tile([128, KB1, ns0], FP8, tag="x0")
        xrt = wpool.tile([128, KB1, max(k_rest, 16)], FP8, tag="xr")
        bt = wpool.tile([128, MC1 + MC2], F32, tag="bias")
        # w2 as four tiles/DMAs: a single dma_start stripes only ~4 of
        # the 16 queues (one quad), and the whole 1MB w2 on one stripe
        # set lands ~4us after chunk 0's mm2 wants it. Four DMAs land on
        # four different quads and drain in parallel.
        w2t = [
            wpool.tile([128, 2, OUT_DIM], BF16, tag=f"w2{i}", name=f"w2t{i}")
            for i in range(4)
        ]
        xwarm = wpool.tile([128, 2, 512], FP8, tag="xwarm")

        # All DMAs issue on the SP (sync) HWDGE: the Activation-engine DGE
        # path measured ~2x slower descriptor processing on the shared
        # queues, slowing every transfer (tried and reverted). Order is
        # first-use order; the warm matmuls below keep the PE busy while
        # the first inputs land.
        # Ladder order trades the ~0.65us serialized DIRECT2D issue cost
        # on the sync sequencer against each consumer's deadline: x0+w1a
        # gate the PE start, w2 quarter 0 must beat chunk 0's mm2 (~3us
        # after PE start), w1b is needed one g-pass in, bias before the
        # first sigmoid, xr only ~9us in (it goes last, as two partition
        # halves to drain on two quads).
        nc.sync.dma_start(w1t[0][:], w1[:, 0:2, :])
        nc.sync.dma_start(x0t[:], eps0[:])
        nc.sync.dma_start(w2t[0][:], w2[:, 0:2, :])
        nc.sync.dma_start(w1t[1][:], w1[:, 2:4, :])
        nc.sync.dma_start(w2t[1][:], w2[:, 2:4, :])
        nc.sync.dma_start(bt[:], bias[:])
        nc.sync.dma_start(w2t[2][:], w2[:, 4:6, :])
        nc.sync.dma_start(w2t[3][:], w2[:, 6:8, :])
        if k_rest:
            nc.sync.dma_start(xrt[0:64], epsr[0:64])
            nc.sync.dma_start(xrt[64:128], epsr[64:128])

        # PE p-state warmup: the array runs ~2x slow until ~3us of
        # continuous execution, so burn that ramp on garbage DoubleRow
        # matmuls over a memset scratch while the real inputs land.
        nc.vector.memset(xwarm[:], 0)
        pswarm = pspool.tile([128, 512], F32, tag="ps", name="pswarm")
        for i in range(N_WARM_MM):
            nc.tensor.matmul(
                pswarm[:], xwarm[:, :, 0:128], xwarm[:], start=True, stop=True,
                perf_mode=DR,
            )
        # Sigmoid table warmup: the first Sigmoid pays ~1.3us of
        # ACT_TABLE_LOAD; hide it in the head (emitted after the scalar
        # engine's DMA ladder so it doesn't delay those issues).
        warm = hpool.tile([128, 1], BF16, tag="warm")
        nc.scalar.activation(warm[:], xwarm[:, 0, 0:1], SIG)

        for ci, (n0, ns) in enumerate(chunks):
            if ci == 0:
                def rhs1(g):
                    return x0t[:, 2 * g : 2 * g + 2, :]
            else:
                r0 = n0 - ns0

                def rhs1(g, r0=r0, ns=ns):
                    return xrt[:, 2 * g : 2 * g + 2, r0 : r0 + ns]

            # mm1: fp8 DoubleRow, contraction 512 = 2 groups x (2 blocks
            # packed per cell x 128 partitions). g-outer so the first pass
            # only needs the g=0 half of w1/eps; each ps1[mc] stops on the
            # g=1 pass and its sigmoid overlaps the rest of mm1.
            ht = []
            ps1 = [
                pspool.tile([128, ns], F32, tag="ps", name=f"ps1_{ci}_{i}")
                for i in range(MC1)
            ]
            for g in range(2):
                for mc in range(MC1):
                    nc.tensor.matmul(
                        ps1[mc][:],
                        w1t[g][:, :, mc * 128 : (mc + 1) * 128],
                        rhs1(g),
                        start=(g == 0),
                        stop=(g == 1),
                        perf_mode=DR,
                    )
                    if g == 1:
                        h = hpool.tile([128, ns], BF16, tag=f"h{mc}")
                        nc.scalar.activation(
                            h[:], ps1[mc][:], SIG,
                            bias=bt[:, mc : mc + 1], scale=1.0 / W1_SCALE,
                        )
                        ht.append(h)

            # mm2: bf16, oc-outer: each ps2[oc] finishes 8 matmuls apart,
            # so bias-add + y writeback stream during mm2 rather than all
            # bunching after it. h[kc] is ready ~0.5us after its mm1 stop,
            # well before the first oc pass reaches it.
            last = ci == len(chunks) - 1
            for oc in range(MC2):
                ps2 = pspool.tile([128, ns], F32, tag="ps", name=f"ps2_{ci}_{oc}")
                for kc in range(KC2):
                    nc.tensor.matmul(
                        ps2[:],
                        w2t[kc // 2][:, kc % 2, oc * 128 : (oc + 1) * 128],
                        ht[kc][:],
                        start=(kc == 0),
                        stop=(kc == KC2 - 1),
                    )
                y = ypool.tile([128, ns], BF16, tag="y")
                nc.vector.tensor_scalar_add(y[:], ps2[:], bt[:, MC1 + oc : MC1 + oc + 1])
                dst = yT[oc * 128 : (oc + 1) * 128, n0 : n0 + ns]
                if last and oc >= MC2 - 2:
                    # the final writebacks are the tail: partition-half DMAs
                    # stripe twice the queues and halve their drain. (Not
                    # done for every write — each extra issue costs ~0.65us
                    # serialized on the sync sequencer.)
                    nc.sync.dma_start(dst[0:64], y[0:64])
                    nc.sync.dma_start(dst[64:128], y[64:128])
                else:
                    nc.sync.dma_start(dst, y[:])

    nc.compile()
    return nc


def get_program(k_cap):
    if k_cap not in _program_cache:
        _program_cache[k_cap] = _build_program(k_cap)
    return _program_cache[k_cap]


def _softplus(x):
    x = x.astype(np.float64)
    return (np.maximum(x, 0.0) + np.log1p(np.exp(-np.abs(x)))).astype(np.float32)


def _pack_blocks(a, nblk):
    """[nblk*128, C] -> [128, nblk, C] with out[p, b, c] = a[b*128+p, c]."""
    return np.ascontiguousarray(
        a.reshape(nblk, 128, a.shape[1]).transpose(1, 0, 2)
    )


def kernel(epsilon, comp_idx, mu, rho, W1, b1, W2, b2, _trace=False):
    epsilon = np.asarray(epsilon, dtype=np.float32)
    comp_idx = np.asarray(comp_idx, dtype=np.int32)
    mu = np.asarray(mu, dtype=np.float32)
    rho = np.asarray(rho, dtype=np.float32)
    W1 = np.asarray(W1, dtype=np.float32)
    b1 = np.asarray(b1, dtype=np.float32)
    W2 = np.asarray(W2, dtype=np.float32)
    b2 = np.asarray(b2, dtype=np.float32)

    n = epsilon.shape[0]
    sigma = _softplus(rho)  # [C]

    sels = [np.nonzero(comp_idx == c)[0] for c in range(NB_COMP)]
    counts = [len(s) for s in sels]
    k_cap = max(256, -(-max(counts) // 16) * 16)

    nc = get_program(k_cap)
    ns0 = _make_chunks(k_cap)[0][1]
    k_rest = k_cap - ns0

    eps_q = epsilon.astype(E4M3)  # quantize once; std ~1 sits mid e4m3 range
    in_maps = []
    for c in range(NB_COMP):
        sel = sels[c]
        epsT = np.zeros((128, KB1, k_cap), dtype=E4M3)
        if len(sel):
            epsT[:, :, : len(sel)] = _pack_blocks(eps_q[sel].T, KB1)
        w1p = (W1[c] * (sigma[c] * W1_SCALE)).astype(E4M3)
        b1p = (
            b1[c].astype(np.float64) + mu[c].astype(np.float64) @ W1[c].astype(np.float64)
        ).astype(np.float32)
        bias_c = np.concatenate(
            [b1p.reshape(MC1, 128).T, b2[c].reshape(MC2, 128).T], axis=1
        )
        in_maps.append(
            {
                "eps0": np.ascontiguousarray(epsT[:, :, :ns0]),
                "epsr": np.ascontiguousarray(epsT[:, :, ns0:])
                if k_rest
                else np.zeros((128, KB1, 16), dtype=E4M3),
                "w1": _pack_blocks(w1p, KB1),
                "w2": _pack_blocks(W2[c].astype(NPBF16), KC2),
                "bias": np.ascontiguousarray(bias_c),
            }
        )

    res = run_bass_kernel_spmd(
        nc,
        in_maps,
        core_ids=list(range(N_CORES)),
        trace=_trace,
        trace_cores=list(range(N_CORES)) if _trace else None,
    )

    out = np.zeros((n, OUT_DIM), dtype=np.float32)
    for c in range(NB_COMP):
        sel = sels[c]
        if len(sel):
            out[sel] = res.results[c]["yT"][:, : len(sel)].T.astype(np.float32)
    if _trace:
        return out, res
    return out


# revision 37
# speedup vs baseline: 1.0698x; 1.0698x over previous
"""GNN ensemble MoE-routing kernel for Trainium2 (8 NeuronCores).

Reference computes all 8 expert MLPs for every sample then selects one
(8x wasted FLOPs). This kernel routes on the host instead: samples are
gathered per expert, and core c runs ONLY expert c's MLP over the
samples routed to it (expert-parallel sharding).

Math folding (exact):
  lat = eps*sigma_c + mu_c  =>  lat @ W1_c = eps @ (sigma_c*W1_c) + mu_c@W1_c
so the device computes  sigmoid(eps @ W1p + b1p) @ W2 + b2  with
  W1p = sigma_c * W1_c,  b1p = b1_c + mu_c @ W1_c  (folded on host).

Precision/perf layout (rel-err budget 2e-2; measured ~1.2e-2):
  mm1 runs in fp8 e4m3 with perf_mode=DoubleRow (2 contraction rows
  packed per PE cell -> half the matmul count of bf16). eps quantizes
  to e4m3 raw (std 1.0); W1p is scaled x64 before quantization so its
  ~0.03-std values stay in e4m3's normal range, and the 1/64 rescale
  folds into the sigmoid activation's scale operand. mm2 runs in bf16
  (same PE speed as f32r, half the DMA bytes). y writes back as bf16.

Device layout: features on SBUF partitions, samples on the free axis.
DoubleRow operands are [128, K_blocks, free] with contraction index
k = block*128 + partition; host tensors pack as
reshape(blocks,128,cols).transpose(1,0,2) -> contiguous multi-KB DMA
descriptors. Chunk 0's eps columns are packed as their own dram
tensor so the first transfer is small and fully contiguous.

Startup hiding (the NEFF spends ~6.5us in a fixed semaphore/iram
preamble, then ~4-5us moving the first inputs):
  - The PE's first 8 real matmuls would run ~2x slow (p-state ramp,
    ~3us to full clock), so 8 throwaway DoubleRow matmuls over a
    memset scratch tile ramp the array while the real inputs land.
  - The first Sigmoid pays a ~1.3us ACT_TABLE_LOAD; a throwaway
    activation preloads the table during the head.
mm1 iterates g-outer (first pass needs only half of w1/eps), mm2
oc-outer so bias-add + writeback stream during mm2.
"""

from contextlib import ExitStack

import ml_dtypes
import numpy as np

import concourse.bass as bass
import concourse.tile as tile
from concourse import bacc, mybir
from concourse.bass_utils import run_bass_kernel_spmd

NB_COMP = 8
LAT_DIM = 512
NB_NEUR = 1024
OUT_DIM = 512
N_CORES = 8

F32 = mybir.dt.float32
BF16 = mybir.dt.bfloat16
FP8 = mybir.dt.float8e4
SIG = mybir.ActivationFunctionType.Sigmoid
DR = mybir.MatmulPerfMode.DoubleRow

E4M3 = ml_dtypes.float8_e4m3
NPBF16 = ml_dtypes.bfloat16
W1_SCALE = 64.0
N_WARM_MM = 8

KB1 = LAT_DIM // 128   # 4 contraction blocks for mm1
MC1 = NB_NEUR // 128   # 8 output tiles for mm1
KC2 = NB_NEUR // 128   # 8 contraction blocks for mm2
MC2 = OUT_DIM // 128   # 4 output tiles for mm2

_program_cache = {}


def _make_chunks(k_cap):
    """Near-equal chunks, multiples of 16, each <=512 (PSUM bank / moving
    dim limit) and >=256 when possible (full-rate floor). (A small last
    chunk to shorten the writeback tail was tried and measured worse.)"""
    n_chunks = -(-k_cap // 512)
    base = (k_cap // n_chunks) // 16 * 16
    sizes = [base] * n_chunks
    sizes[0] += k_cap - base * n_chunks
    chunks = []
    n0 = 0
    for ns in sizes:
        chunks.append((n0, ns))
        n0 += ns
    return chunks


def _build_program(k_cap):
    """One-expert MLP over k_cap samples; same program runs SPMD on all 8 cores."""
    chunks = _make_chunks(k_cap)
    ns0 = chunks[0][1]
    k_rest = k_cap - ns0

    nc = bacc.Bacc(
        "TRN2",
        target_bir_lowering=False,
        debug=False,
        enable_asserts=False,
        num_devices=N_CORES,
    )
    eps0 = nc.dram_tensor("eps0", [128, KB1, ns0], FP8, kind="ExternalInput").ap()
    epsr = nc.dram_tensor("epsr", [128, KB1, max(k_rest, 16)], FP8, kind="ExternalInput").ap()
    w1 = nc.dram_tensor("w1", [128, KB1, NB_NEUR], FP8, kind="ExternalInput").ap()
    w2 = nc.dram_tensor("w2", [128, KC2, OUT_DIM], BF16, kind="ExternalInput").ap()
    bias = nc.dram_tensor("bias", [128, MC1 + MC2], F32, kind="ExternalInput").ap()
    yT = nc.dram_tensor("yT", [OUT_DIM, k_cap], BF16, kind="ExternalOutput").ap()

    with tile.TileContext(nc) as tc, ExitStack() as ctx:
        wpool = ctx.enter_context(tc.tile_pool(name="weights", bufs=1))
        hpool = ctx.enter_context(tc.tile_pool(name="h", bufs=2))
        ypool = ctx.enter_context(tc.tile_pool(name="y", bufs=10))
        # One shared pool holding all 8 PSUM banks; mm1 keeps 8 accumulators
        # live, mm2 4, cycling through the same slots.
        pspool = ctx.enter_context(tc.tile_pool(name="ps", bufs=8, space="PSUM"))

        w1t = [
            wpool.tile([128, 2, NB_NEUR], FP8, tag=f"w1{g}", name=f"w1t{g}")
            for g in range(2)
        ]
        x0t = wpool.tile([128, KB1, ns0], FP8, tag="x0")
        xrt = wpool.tile([128, KB1, max(k_rest, 16)], FP8, tag="xr")
        bt = wpool.tile([128, MC1 + MC2], F32, tag="bias")
        # w2 as four tiles/DMAs: a single dma_start stripes only ~4 of
        # the 16 queues (one quad), and the whole 1MB w2 on one stripe
        # set lands ~4us after chunk 0's mm2 wants it. Four DMAs land on
        # four different quads and drain in parallel.
        w2t = [
            wpool.tile([128, 2, OUT_DIM], BF16, tag=f"w2{i}", name=f"w2t{i}")
            for i in range(4)
        ]
        xwarm = wpool.tile([128, 2, 512], FP8, tag="xwarm")

        # All DMAs issue on the SP (sync) HWDGE: the Activation-engine DGE
        # path measured ~2x slower descriptor processing on the shared
        # queues, slowing every transfer (tried and reverted). Order is
        # first-use order; the warm matmuls below keep the PE busy while
        # the first inputs land.
        # Ladder order trades the ~0.65us serialized DIRECT2D issue cost
        # on the sync sequencer against each consumer's deadline: x0+w1a
        # gate the PE start, w2 quarter 0 must beat chunk 0's mm2 (~3us
        # after PE start), w1b is needed one g-pass in, bias before the
        # first sigmoid, xr only ~9us in (it goes last, as two partition
        # halves to drain on two quads).
        nc.sync.dma_start(x0t[:], eps0[:])
        nc.sync.dma_start(w1t[0][:], w1[:, 0:2, :])
        nc.sync.dma_start(w2t[0][:], w2[:, 0:2, :])
        nc.sync.dma_start(w1t[1][:], w1[:, 2:4, :])
        nc.sync.dma_start(w2t[1][:], w2[:, 2:4, :])
        nc.sync.dma_start(bt[:], bias[:])
        nc.sync.dma_start(w2t[2][:], w2[:, 4:6, :])
        nc.sync.dma_start(w2t[3][:], w2[:, 6:8, :])
        if k_rest:
            nc.sync.dma_start(xrt[0:64], epsr[0:64])
            nc.sync.dma_start(xrt[64:128], epsr[64:128])

        # PE p-state warmup: the array runs ~2x slow until ~3us of
        # continuous execution, so burn that ramp on garbage DoubleRow
        # matmuls over a memset scratch while the real inputs land.
        nc.vector.memset(xwarm[:], 0)
        pswarm = pspool.tile([128, 512], F32, tag="ps", name="pswarm")
        for i in range(N_WARM_MM):
            nc.tensor.matmul(
                pswarm[:], xwarm[:, :, 0:128], xwarm[:], start=True, stop=True,
                perf_mode=DR,
            )
        # Sigmoid table warmup: the first Sigmoid pays ~1.3us of
        # ACT_TABLE_LOAD; hide it in the head (emitted after the scalar
        # engine's DMA ladder so it doesn't delay those issues).
        warm = hpool.tile([128, 1], BF16, tag="warm")
        nc.scalar.activation(warm[:], xwarm[:, 0, 0:1], SIG)

        for ci, (n0, ns) in enumerate(chunks):
            if ci == 0:
                def rhs1(g):
                    return x0t[:, 2 * g : 2 * g + 2, :]
            else:
                r0 = n0 - ns0

                def rhs1(g, r0=r0, ns=ns):
                    return xrt[:, 2 * g : 2 * g + 2, r0 : r0 + ns]

            # mm1: fp8 DoubleRow, contraction 512 = 2 groups x (2 blocks
            # packed per cell x 128 partitions). g-outer so the first pass
            # only needs the g=0 half of w1/eps; each ps1[mc] stops on the
            # g=1 pass and its sigmoid overlaps the rest of mm1.
            ht = []
            ps1 = [
                pspool.tile([128, ns], F32, tag="ps", name=f"ps1_{ci}_{i}")
                for i in range(MC1)
            ]
            for g in range(2):
                for mc in range(MC1):
                    nc.tensor.matmul(
                        ps1[mc][:],
                        w1t[g][:, :, mc * 128 : (mc + 1) * 128],
                        rhs1(g),
                        start=(g == 0),
                        stop=(g == 1),
                        perf_mode=DR,
                    )
                    if g == 1:
                        h = hpool.tile([128, ns], BF16, tag=f"h{mc}")
                        nc.scalar.activation(
                            h[:], ps1[mc][:], SIG,
                            bias=bt[:, mc : mc + 1], scale=1.0 / W1_SCALE,
                        )
                        ht.append(h)

            # mm2: bf16, oc-outer: each ps2[oc] finishes 8 matmuls apart,
            # so bias-add + y writeback stream during mm2 rather than all
            # bunching after it. h[kc] is ready ~0.5us after its mm1 stop,
            # well before the first oc pass reaches it.
            last = ci == len(chunks) - 1
            for oc in range(MC2):
                ps2 = pspool.tile([128, ns], F32, tag="ps", name=f"ps2_{ci}_{oc}")
                for kc in range(KC2):
                    nc.tensor.matmul(
                        ps2[:],
                        w2t[kc // 2][:, kc % 2, oc * 128 : (oc + 1) * 128],
                        ht[kc][:],
                        start=(kc == 0),
                        stop=(kc == KC2 - 1),
                    )
                y = ypool.tile([128, ns], BF16, tag="y")
                nc.vector.tensor_scalar_add(y[:], ps2[:], bt[:, MC1 + oc : MC1 + oc + 1])
                dst = yT[oc * 128 : (oc + 1) * 128, n0 : n0 + ns]
                if last and oc >= MC2 - 2:
                    # the final writebacks are the tail: partition-half DMAs
                    # stripe twice the queues and halve their drain. (Not
                    # done for every write — each extra issue costs ~0.65us
                    # serialized on the sync sequencer.)
                    nc.sync.dma_start(dst[0:64], y[0:64])
                    nc.sync.dma_start(dst[64:128], y[64:128])
                else:
                    nc.sync.dma_start(dst, y[:])

    nc.compile()
    return nc


def get_program(k_cap):
    if k_cap not in _program_cache:
        _program_cache[k_cap] = _build_program(k_cap)
    return _program_cache[k_cap]


def _softplus(x):
    x = x.astype(np.float64)
    return (np.maximum(x, 0.0) + np.log1p(np.exp(-np.abs(x)))).astype(np.float32)


def _pack_blocks(a, nblk):
    """[nblk*128, C] -> [128, nblk, C] with out[p, b, c] = a[b*128+p, c]."""
    return np.ascontiguousarray(
        a.reshape(nblk, 128, a.shape[1]).transpose(1, 0, 2)
    )


def kernel(epsilon, comp_idx, mu, rho, W1, b1, W2, b2, _trace=False):
    epsilon = np.asarray(epsilon, dtype=np.float32)
    comp_idx = np.asarray(comp_idx, dtype=np.int32)
    mu = np.asarray(mu, dtype=np.float32)
    rho = np.asarray(rho, dtype=np.float32)
    W1 = np.asarray(W1, dtype=np.float32)
    b1 = np.asarray(b1, dtype=np.float32)
    W2 = np.asarray(W2, dtype=np.float32)
    b2 = np.asarray(b2, dtype=np.float32)

    n = epsilon.shape[0]
    sigma = _softplus(rho)  # [C]

    sels = [np.nonzero(comp_idx == c)[0] for c in range(NB_COMP)]
    counts = [len(s) for s in sels]
    k_cap = max(256, -(-max(counts) // 16) * 16)

    nc = get_program(k_cap)
    ns0 = _make_chunks(k_cap)[0][1]
    k_rest = k_cap - ns0

    eps_q = epsilon.astype(E4M3)  # quantize once; std ~1 sits mid e4m3 range
    in_maps = []
    for c in range(NB_COMP):
        sel = sels[c]
        epsT = np.zeros((128, KB1, k_cap), dtype=E4M3)
        if len(sel):
            epsT[:, :, : len(sel)] = _pack_blocks(eps_q[sel].T, KB1)
        w1p = (W1[c] * (sigma[c] * W1_SCALE)).astype(E4M3)
        b1p = (
            b1[c].astype(np.float64) + mu[c].astype(np.float64) @ W1[c].astype(np.float64)
        ).astype(np.float32)
        bias_c = np.concatenate(
            [b1p.reshape(MC1, 128).T, b2[c].reshape(MC2, 128).T], axis=1
        )
        in_maps.append(
            {
                "eps0": np.ascontiguousarray(epsT[:, :, :ns0]),
                "epsr": np.ascontiguousarray(epsT[:, :, ns0:])
                if k_rest
                else np.zeros((128, KB1, 16), dtype=E4M3),
                "w1": _pack_blocks(w1p, KB1),
                "w2": _pack_blocks(W2[c].astype(NPBF16), KC2),
                "bias": np.ascontiguousarray(bias_c),
            }
        )

    res = run_bass_kernel_spmd(
        nc,
        in_maps,
        core_ids=list(range(N_CORES)),
        trace=_trace,
        trace_cores=list(range(N_CORES)) if _trace else None,
    )

    out = np.zeros((n, OUT_DIM), dtype=np.float32)
    for c in range(NB_COMP):
        sel = sels[c]
        if len(sel):
            out[sel] = res.results[c]["yT"][:, : len(sel)].T.astype(np.float32)
    if _trace:
        return out, res
    return out
